# revision 1
# baseline (speedup 1.0000x reference)
"""AttentionGNN Trainium kernel — data-parallel over B=256 graphs on 8 NeuronCores.

Sharding: 32 graphs (2048 nodes, 16384 edges) per core, all weights replicated;
segment softmax and dense attention are fully graph-local, so no collectives.

Key restructurings vs the reference (math-equivalent, hardware-friendly):
  - src = repeat(arange(N), 8) is static -> segment ops are reshapes to [.., 64, 8].
  - The [E, H] edge embedding is only consumed via (e @ a_l[2H:]); collapsed to
    edge_feats @ (We @ a_l[2H:]) + be . a_l[2H:]  -> an [E, 32] @ [32, 4] matmul.
  - Per-edge gathers (s2[dst], m[dst]) are expressed through a one-hot of the
    graph-local dst (dst % 64), turning the scatter/gather segment-sum into
    dense per-graph matmuls that map onto the TensorEngine.
"""

import numpy as np
import jax
import jax.numpy as jnp
from functools import partial

B, NPG, DEG = 256, 64, 8
N, E = B * NPG, B * NPG * DEG
NODE_IN, EDGE_IN, H, L, HEADS = 64, 32, 256, 4, 8
NCORES = 8
BL = B // NCORES            # graphs per core
NL, EL_ = BL * NPG, BL * NPG * DEG


def _ln(x, g, b, eps):
    mu = jnp.mean(x, axis=-1, keepdims=True)
    var = jnp.mean((x - mu) ** 2, axis=-1, keepdims=True)
    return (x - mu) / jnp.sqrt(var + eps) * g + b


def _local(node_feats, edge_feats, dst_local, wea, bedot,
           Wn, bn, gat_W, gat_a, gat_lng, gat_lnb,
           Wq, Wk, Wv, att_lng, att_lnb,
           ff_W1, ff_b1, ff_W2, ff_b2, ff_lng, ff_lnb,
           g_W1, g_b1, g_W2, g_b2):
    """Per-core computation. node_feats [NL,64], edge_feats [EL,32],
    dst1h [BL,64,8,64] one-hot of graph-local dst."""
    dst1h = jax.nn.one_hot(dst_local.reshape(BL, NPG, DEG), NPG,
                           dtype=jnp.float32)                  # [BL,64,8,64]
    h = node_feats @ Wn + bn                                   # [NL, H]
    # all-layer edge logit contributions: [EL, L]
    el_all = edge_feats @ wea + bedot                          # [EL, L]
    el_all = el_all.reshape(BL, NPG, DEG, L)

    for i in range(L):
        m = h @ gat_W[i]                                       # [NL, H]
        a = gat_a[i]
        s12 = m @ a[:2 * H].reshape(2, H).T                    # [NL, 2]
        s1, s2 = s12[:, 0], s12[:, 1]
        mg = m.reshape(BL, NPG, H)
        s2g = s2.reshape(BL, NPG)
        # gather s2 at dst via one-hot matmul: [BL,64,8]
        s2d = jnp.einsum('bnkj,bj->bnk', dst1h, s2g)
        logits = s1.reshape(BL, NPG, 1) + s2d + el_all[..., i]
        logits = jnp.where(logits >= 0, logits, 0.01 * logits)
        mx = jnp.max(logits, axis=-1, keepdims=True)
        ex = jnp.exp(logits - mx)
        attn = ex / jnp.sum(ex, axis=-1, keepdims=True)        # [BL,64,8]
        # attention-weighted adjacency: Aw[b,n,j] = sum_k attn[b,n,k] 1[dst=j]
        Aw = jnp.einsum('bnk,bnkj->bnj', attn, dst1h)          # [BL,64,64]
        agg = jnp.einsum('bnj,bjh->bnh', Aw, mg)               # [BL,64,H]
        h = _ln(agg.reshape(NL, H) + h, gat_lng[i], gat_lnb[i], 1e-5)

    x = h.reshape(BL, NPG, H)
    dk = H // HEADS
    def split(t):
        return t.reshape(BL, NPG, HEADS, dk).transpose(0, 2, 1, 3)
    qkv = x @ jnp.concatenate([Wq, Wk, Wv], axis=1)            # [BL,64,3H]
    q, k, v = (split(qkv[..., j * H:(j + 1) * H]) for j in range(3))
    scores = jnp.einsum('bhqd,bhkd->bhqk', q, k) / np.float32(np.sqrt(dk))
    scores = jax.nn.softmax(scores, axis=-1)
    o = jnp.einsum('bhqk,bhkd->bhqd', scores, v).transpose(0, 2, 1, 3).reshape(BL, NPG, H)
    x = _ln(o + x, att_lng, att_lnb, 1e-6)

    y = jax.nn.gelu(x @ ff_W1 + ff_b1, approximate=False) @ ff_W2 + ff_b2
    x = _ln(x + y, ff_lng, ff_lnb, 1e-6)

    g = jax.nn.relu(x @ g_W1 + g_b1) @ g_W2 + g_b2             # [BL, NPG]
    g = jax.nn.softmax(g, axis=1)
    return jnp.sum(x * g[..., None], axis=1)                   # [BL, H]


_PMAPPED = None


def _get_pmapped():
    global _PMAPPED
    if _PMAPPED is None:
        _PMAPPED = jax.pmap(
            _local,
            in_axes=(0, 0, 0) + (None,) * 23,
            devices=jax.devices()[:NCORES],
        )
    return _PMAPPED


def kernel(node_feats, edge_feats, src, dst, Wn, bn, We, be,
           gat_W, gat_a, gat_lng, gat_lnb,
           Wq, Wk, Wv, att_lng, att_lnb,
           ff_W1, ff_b1, ff_W2, ff_b2, ff_lng, ff_lnb,
           g_W1, g_b1, g_W2, g_b2):
    node_feats = np.asarray(node_feats, np.float32)
    edge_feats = np.asarray(edge_feats, np.float32)
    dst = np.asarray(dst)

    # host-side prep (cheap): graph-local dst, collapsed edge weights
    dst_local = (dst.astype(np.int64) % NPG).astype(np.int32)
    dst_sh = dst_local.reshape(NCORES, EL_)

    gat_a = np.asarray(gat_a, np.float32)
    We = np.asarray(We, np.float32)
    be = np.asarray(be, np.float32)
    wea = We @ gat_a[:, 2 * H:].T                              # [32, L]
    bedot = be @ gat_a[:, 2 * H:].T                            # [L]

    nf = node_feats.reshape(NCORES, NL, NODE_IN)
    ef = edge_feats.reshape(NCORES, EL_, EDGE_IN)

    fn = _get_pmapped()
    with jax.default_matmul_precision("bfloat16"):
        out = fn(nf, ef, dst_sh, wea, bedot,
             Wn, bn, gat_W, gat_a, gat_lng, gat_lnb,
             Wq, Wk, Wv, att_lng, att_lnb,
             ff_W1, ff_b1, ff_W2, ff_b2, ff_lng, ff_lnb,
                 g_W1, g_b1, g_W2, g_b2)
    return np.asarray(out).reshape(B, H).astype(np.float32)

